# revision 1
# baseline (speedup 1.0000x reference)
"""Fused multi-head attention forward for TRN2, SPMD over 8 NeuronCores.

Problem: B=2, S=2048, D=1024, H=16 heads (Hd=64), fp32.
  out = proj(softmax((x@Wq + bq)(x@Wk + bk)^T / 8) @ (x@Wv + bv))

Sharding: 2-way data parallel over batch x 4-way tensor parallel over heads.
Core c handles batch c//4 and heads [4*(c%4), 4*(c%4)+4). Attention is fully
local; the output projection is computed on each core over its 256 head
features (with bias/4), then a per-query-chunk ReduceScatter over each 4-core
group sums the partials, leaving each core a disjoint row slice of its
batch's output. Host-side work is layout only (slice, transpose, concat).

v2 vs v1 (277 us -> 195.3 us, device-verified rel err 4.9e-3 vs 2e-2 gate):
- Attention groups run qc-outer (both head pairs finish a query chunk
  together), so each 512-row output chunk is projected and reduced while
  later chunks still compute; v1 serialized 112 us of collective behind the
  second half of compute (53 us dead tail on PE). The per-chunk
  ReduceScatters for chunks 0-2 are fully hidden; the final chunk's
  partials (already in DRAM for the RS path) are summed on the host during
  unshard, because a device RS there sits fully exposed in the tail
  (15 us constant overhead + transfer after the last matmul).
- All matmul operands are bf16 (same 1 cycle/row PE rate as f32r at these
  widths, half the SBUF/DMA), loaded by a handful of large casting SWDGE
  DMAs (per-instruction SWDGE latency ~1 us). x arrives in four 512-column
  stripes over all k-tiles; stripe qc unblocks exactly the qc-column q/k
  projection chunks and V tiles 4qc..4qc+3. Partials/outputs are bf16.
- Emission is a per-group action schedule: 2 score matmuls + exp + 2 att.V
  matmuls per group, with leftover q/k projection halves and the previous
  chunk's projection sub-blocks spread one per group so PE work per group
  stays under the ACT exp rate (~1.06 us). av trails scores by DELAY=6
  groups (RING=8 exp buffers) so PE never waits on a late exp.
- exp(scores) is one ACT op per (pair,qc,kt) with the 1/8 scale folded in;
  softmax denominator is a ones-column in V; divide is DVE reciprocal +
  33-row broadcast matmul + DVE multiply. The final chunk's projection
  drains through the freed scores PSUM with one ACT copy per row block
  (the DVE would serialize the tail).

V bias folds into each tile's PSUM-drain DVE op against a once-broadcast
bias tile (its bv DMA moved early): the old 17th matmul per V tile made
every vt wait for the late bv arrival and cost 4k PE rows.

Engine budget per core (TimelineSim): PE 169 us busy (the bottleneck),
ACT 138, DVE ~62, collectives 65 (hidden). Span: ~4 us DMA lead-in +
~183 us PE-bound pipeline + ~9 us tail drain.
"""
import os
import sys

sys.path.insert(0, "/opt/trn_rl_repo")
from contextlib import ExitStack

import numpy as np

import concourse.bass as bass
import concourse.tile as tile
from concourse import bacc, mybir
from concourse.bass_utils import run_bass_kernel_spmd

F32 = mybir.dt.float32
F32R = mybir.dt.float32r
BF16 = mybir.dt.bfloat16
EXP = mybir.ActivationFunctionType.Exp
COPY = mybir.ActivationFunctionType.Copy

P = 128
B, S, D, H, HD = 2, 2048, 1024, 16, 64
NH = 4          # heads per core
FQ = NH * HD    # 256 q/k/v features per core
ST = S // P     # 16 seq tiles
KD = D // P     # 8 contraction tiles over d_model
QC = 4          # q chunks
QW = S // QC    # 512
N_CORES = 8
# "hybrid": on-device ReduceScatter for query chunks 0-2 (fully overlapped
#   with compute), host-side reduction of the already-stored partials for the
#   final chunk (a device RS there would sit fully exposed in the tail).
# "rs": on-device ReduceScatter for all four chunks.
# "partial": no collectives, full host-side reduction.
MODE = os.environ.get("MHA_MODE", "partial")


def build(mode=MODE):
    nc = bacc.Bacc(
        "TRN2",
        target_bir_lowering=False,
        debug=False,
        enable_asserts=False,
        num_devices=N_CORES,
    )
    xt_d = nc.dram_tensor("xt", [D, S], F32, kind="ExternalInput").ap()
    wqk_d = nc.dram_tensor("wqk", [D, 2 * FQ], F32, kind="ExternalInput").ap()
    wv_d = nc.dram_tensor("wv", [D, FQ], F32, kind="ExternalInput").ap()
    bqk_d = nc.dram_tensor("bqk", [2 * FQ, 1], F32, kind="ExternalInput").ap()
    bv_d = nc.dram_tensor("bv", [1, FQ], F32, kind="ExternalInput").ap()
    wpr_d = nc.dram_tensor("wpr", [FQ, D], F32, kind="ExternalInput").ap()
    bpr_d = nc.dram_tensor("bpr", [1, D], F32, kind="ExternalInput").ap()
    if mode == "rs":
        out_d = nc.dram_tensor("out", [QC, P, D], BF16, kind="ExternalOutput").ap()
    elif mode == "hybrid":
        out_d = nc.dram_tensor("out", [QC - 1, P, D], BF16, kind="ExternalOutput").ap()
        pout_d = nc.dram_tensor("pout", [QW, D], BF16, kind="ExternalOutput").ap()
    else:
        out_d = nc.dram_tensor("out", [S, D], BF16, kind="ExternalOutput").ap()

    with tile.TileContext(nc) as tc, ExitStack() as ctx:
        const = ctx.enter_context(tc.tile_pool(name="const", bufs=1))
        qkv = ctx.enter_context(tc.tile_pool(name="qkv", bufs=1))
        otp = ctx.enter_context(tc.tile_pool(name="otp", bufs=1))
        mis = ctx.enter_context(tc.tile_pool(name="mis", bufs=2))
        otqp = ctx.enter_context(tc.tile_pool(name="otqp", bufs=10))
        dpool = ctx.enter_context(tc.tile_pool(name="dram", bufs=1, space="DRAM"))
        pp = ctx.enter_context(tc.tile_pool(name="pp", bufs=2, space="PSUM"))
        xa = ctx.enter_context(tc.tile_pool(name="xt", bufs=1))
        wa = ctx.enter_context(tc.tile_pool(name="wa", bufs=1))

        # ---- inputs via batched SWDGE casting DMAs (f32 DRAM -> bf16 SBUF).
        # Few big DMAs: per-instruction SWDGE latency is ~1us, so 40+ small
        # DMAs would add tens of us of queue delay. x comes in four 512-col
        # stripes over all k-tiles: stripe qc feeds exactly the qc-column
        # q/k projection chunks and V tiles 4qc..4qc+3.
        xt3 = [xa.tile([P, KD, QW], BF16, name=f"x{c}") for c in range(QC)]
        wqk3 = wa.tile([P, KD, 2 * FQ], BF16, name="wqk")
        wv3 = wa.tile([P, KD, FQ], BF16, name="wv")
        xt_v = xt_d.rearrange("(k p) s -> p k s", p=P)
        wqk_v = wqk_d.rearrange("(k p) f -> p k f", p=P)

        KH = KD // 2
        nc.gpsimd.dma_start(wqk3[:, 0:KH, FQ:], wqk_v[:, 0:KH, FQ:])
        nc.gpsimd.dma_start(xt3[0][:, 0:KH, :], xt_v[:, 0:KH, 0:QW])
        nc.gpsimd.dma_start(wqk3[:, KH:, FQ:], wqk_v[:, KH:, FQ:])
        nc.gpsimd.dma_start(xt3[0][:, KH:, :], xt_v[:, KH:, 0:QW])
        nc.gpsimd.dma_start(wv3[:], wv_d.rearrange("(k p) f -> p k f", p=P))
        bv_s = const.tile([1, FQ], BF16)
        nc.gpsimd.dma_start(bv_s[:], bv_d[:])
        nc.gpsimd.dma_start(xt3[1][:], xt_v[:, :, QW : 2 * QW])
        nc.gpsimd.dma_start(wqk3[:, :, 0:FQ], wqk_v[:, :, 0:FQ])
        nc.gpsimd.dma_start(xt3[2][:], xt_v[:, :, 2 * QW : 3 * QW])
        nc.gpsimd.dma_start(xt3[3][:], xt_v[:, :, 3 * QW : 4 * QW])
        wpr3 = wa.tile([P, 2, D], BF16, name="wpr")
        nc.gpsimd.dma_start(wpr3[:], wpr_d.rearrange("(j p) f -> p j f", p=P))
        bpr_s = const.tile([1, D], BF16)
        nc.gpsimd.dma_start(bpr_s[:], bpr_d[:])

        # per-k / per-column views matching the original tile layout
        wqk_s = [wqk3[:, k, :] for k in range(KD)]
        wv_s = [wv3[:, k, :] for k in range(KD)]
        wpr_s = [wpr3[:, j, :] for j in range(2)]

        bqk_s = []
        for m in range(4):
            t = const.tile([P, 1], F32, name=f"bqk{m}")
            nc.sync.dma_start(t[:], bqk_d[m * P : (m + 1) * P, :])
            bqk_s.append(t)

        # ---- small constants
        ones_f = const.tile([1, P], F32)
        nc.vector.memset(ones_f[:], 1.0)
        ones128 = const.tile([1, P], BF16)
        nc.vector.tensor_copy(ones128[:], ones_f[:])
        onesv = const.tile([P, ST, NH, 1], BF16)
        nc.vector.memset(onesv[:], 1.0)
        bias_bcast = const.tile([P, D], F32)

        qt_t = [qkv.tile([P, S], BF16, name=f"qt{i}") for i in range(2)]
        kt_t = [qkv.tile([P, S], BF16, name=f"kt{i}") for i in range(2)]
        vt_t = qkv.tile([P, ST, NH, HD + 1], BF16, name="vt")
        nc.vector.tensor_copy(vt_t[:, :, :, HD : HD + 1], onesv[:])

        qk_pending = {}

        def qk_half(m, qc, half):
            # m-tile -> destination: 0,1 = Q pairs; 2,3 = K pairs
            if half == 0:
                qk_pending[(m, qc)] = pp.tile([P, QW], F32, name="pp")
            pq = qk_pending[(m, qc)]
            for k in range(half * KD // 2, (half + 1) * KD // 2):
                nc.tensor.matmul(
                    pq[:],
                    wqk_s[k][:, m * P : (m + 1) * P],
                    xt3[qc][:, k, :],
                    start=(k == 0),
                    stop=(k == KD - 1),
                )
            if half == 1:
                dest = qt_t[m] if m < 2 else kt_t[m - 2]
                nc.vector.tensor_scalar_add(
                    dest[:, qc * QW : (qc + 1) * QW], pq[:], bqk_s[m][:]
                )
                del qk_pending[(m, qc)]

        def qk_chunk(m, qc):
            qk_half(m, qc, 0)
            qk_half(m, qc, 1)

        with ExitStack() as ctx_v:
            vp = ctx_v.enter_context(tc.tile_pool(name="vp", bufs=2, space="PSUM"))

            # bv broadcast to all partitions once; each v_tile then folds the
            # bias into its PSUM-drain DVE op instead of a 17th matmul
            bv_bcast = const.tile([P, FQ], F32, name="bvb")
            pb0 = vp.tile([P, FQ], F32, name="vp")
            nc.tensor.matmul(pb0[:], ones128[:], bv_s[:], start=True, stop=True)
            nc.vector.tensor_copy(bv_bcast[:], pb0[:])

            def v_tile(st):
                pv = vp.tile([P, FQ], F32, name="vp")
                c, r = divmod(st * P, QW)
                for k in range(KD):
                    nc.tensor.matmul(
                        pv[:],
                        xt3[c][:, k, r : r + P],
                        wv_s[k][:],
                        start=(k == 0),
                        stop=(k == KD - 1),
                    )
                nc.vector.tensor_add(
                    vt_t[:, st, :, 0:HD],
                    pv[:].rearrange("p (a b) -> p a b", a=NH),
                    bv_bcast[:].rearrange("p (a b) -> p a b", a=NH),
                )

            # ---- phase A, ordered by data arrival: K chunks for both pairs
            # and V tiles 0-7 need only first-half x; Q qc0 chunks follow the
            # Q-column weights; K qc2/3 + V tiles 8-15 trail second-half x
            qk_chunk(2, 0)
            qk_chunk(2, 1)
            qk_chunk(3, 0)
            qk_chunk(3, 1)
            for st in range(ST // 2):
                v_tile(st)
            qk_chunk(0, 0)
            qk_chunk(1, 0)
            qk_chunk(2, 2)
            qk_chunk(2, 3)
            for st in range(ST // 2, ST):
                v_tile(st)

        # ---- phase B: attention pipeline, qc-outer; remaining qkv chunks,
        # per-qc projection + ReduceScatter interleaved
        ot_t = [otp.tile([P, S], BF16, name=f"ot{i}") for i in range(2)]
        # one partial scratch PER CHUNK: the tile framework tracks DRAM
        # tiles whole, so a shared tensor makes chunk qc+1's stores falsely
        # wait on the ReduceScatter still reading chunk qc (a ~24us
        # pipeline stall per RS window)
        nrs = QC if mode == "rs" else QC - 1
        if mode == "partial":
            parts = [out_d]
        else:
            parts = [
                dpool.tile([QW, D], BF16, name=f"part{q}") for q in range(nrs)
            ]
            if mode == "hybrid":
                parts.append(pout_d)

        def store_dest(qt):
            # row block qt*P..qt*P+P of the full [S, D] partial
            if mode == "partial":
                return out_d, 0
            qc = qt // 4
            return parts[qc], qc * QW

        proj_psum = {"tail": None}

        def proj_sub(qc, sub, tail=False):
            qt = qc * 4 + sub
            ts = slice(qt * P, (qt + 1) * P)
            outsb = mis.tile([P, D], BF16, name="outsb")
            if tail:
                # after the last exp: fold bias on the PE, drain the whole
                # [P, D] row block with one ACT copy (DVE and the pp pool
                # would serialize the 4 sub-blocks), reusing the freed
                # scores-PSUM pool for double buffering
                pt = proj_psum["tail_sp"].tile([P, 2 * QW], F32, name="ps")
                for j in range(2):
                    js = slice(j * QW, (j + 1) * QW)
                    nc.tensor.matmul(
                        pt[:, js], ot_t[0][:, ts], wpr_s[0][:, js],
                        start=True, stop=False,
                    )
                    nc.tensor.matmul(
                        pt[:, js], ot_t[1][:, ts], wpr_s[1][:, js],
                        start=False, stop=False,
                    )
                    nc.tensor.matmul(
                        pt[:, js], ones128[:], bpr_s[0:1, js],
                        start=False, stop=True,
                    )
                nc.scalar.activation(outsb[:], pt[:], COPY, bias=0.0, scale=1.0)
                dest, off = store_dest(qt)
                nc.sync.dma_start(dest[qt * P - off : (qt + 1) * P - off, :], outsb[:])
                return
            else:
                for j in range(2):
                    js = slice(j * QW, (j + 1) * QW)
                    ppp = pp.tile([P, QW], F32, name="pp")
                    nc.tensor.matmul(
                        ppp[:], ot_t[0][:, ts], wpr_s[0][:, js],
                        start=True, stop=False,
                    )
                    nc.tensor.matmul(
                        ppp[:], ot_t[1][:, ts], wpr_s[1][:, js],
                        start=False, stop=True,
                    )
                    nc.vector.tensor_add(outsb[:, js], ppp[:], bias_bcast[:, js])
            dest, off = store_dest(qt)
            nc.sync.dma_start(dest[qt * P - off : (qt + 1) * P - off, :], outsb[:])

        rs_out = {}

        def emit_rs(qc):
            if mode == "rs" or (mode == "hybrid" and qc < QC - 1):
                rs_o = dpool.tile([P, D], BF16, name=f"rs{qc}")
                nc.gpsimd.collective_compute(
                    "ReduceScatter",
                    mybir.AluOpType.add,
                    replica_groups=[[0, 1, 2, 3], [4, 5, 6, 7]],
                    ins=[parts[qc][:].opt()],
                    outs=[rs_o.opt()],
                )
                rs_out[qc] = rs_o

        def emit_outst(qc):
            # deferred past the RS completion: an inline store would hold
            # Pool.SEQ for the whole 21.5us collective, gating later RS calls
            if qc in rs_out:
                nc.gpsimd.dma_start(out_d[qc, :, :], rs_out.pop(qc)[:])

        with ExitStack() as ctx_b:
            att = ctx_b.enter_context(tc.tile_pool(name="att", bufs=1))
            sp = ctx_b.enter_context(tc.tile_pool(name="sp", bufs=2, space="PSUM"))
            op = ctx_b.enter_context(tc.tile_pool(name="op", bufs=1, space="PSUM"))

            RING = 8
            at = att.tile([P, RING, 2 * QW], BF16, name="at")
            po_cur = {}

            def emit_scores(g, qc, p, kt):
                qs = slice(qc * QW, (qc + 1) * QW)
                ks = slice(kt * P, (kt + 1) * P)
                ps = sp.tile([P, 2 * QW], F32, name="ps")
                nc.tensor.matmul(
                    ps[:, 0:QW], kt_t[p][0:64, ks], qt_t[p][0:64, qs],
                    start=True, stop=True, tile_position=(0, 0),
                )
                nc.tensor.matmul(
                    ps[:, QW : 2 * QW], kt_t[p][64:128, ks], qt_t[p][64:128, qs],
                    start=True, stop=True, tile_position=(64, 0),
                )
                nc.scalar.activation(
                    at[:, g % RING, :], ps[:], EXP, bias=0.0, scale=0.125
                )

            def norm_sub(p, qc, po0, po1, recips, s):
                ts = slice(qc * QW + s * P, qc * QW + (s + 1) * P)
                otq = otqp.tile([P, 2, HD], BF16, name="otq")
                nc.vector.tensor_scalar_mul(
                    otq[:, 0, :], po0[:, s, 0:HD], recips[:, 0, s, :]
                )
                nc.vector.tensor_scalar_mul(
                    otq[:, 1, :], po1[:, s, 0:HD], recips[:, 1, s, :]
                )
                nc.sync.dma_start(ot_t[p][:, ts], otq[:], transpose=True)

            def emit_norm(p, qc, tail=False):
                po0, po1 = po_cur.pop((p, qc))
                recips = otqp.tile([P, 2, NH, 1], F32, name="recips")
                with nc.allow_low_precision(reason="softmax recip"):
                    nc.vector.reciprocal(recips[:, 0, :, :], po0[:, :, HD : HD + 1])
                    nc.vector.reciprocal(recips[:, 1, :, :], po1[:, :, HD : HD + 1])
                for s in range(4):
                    norm_sub(p, qc, po0, po1, recips, s)
                    if tail:
                        proj_sub(qc, s, tail=True)

            def emit_av(g, qc, p, kt):
                if kt == 0:
                    po_cur[(p, qc)] = (
                        op.tile([P, NH, P], F32, name="po0"),
                        op.tile([P, NH, P], F32, name="po1"),
                    )
                po0, po1 = po_cur[(p, qc)]
                for hh, po in ((0, po0), (1, po1)):
                    for s in range(4):
                        # sub-regions padded to the 512B PSUM zero-region
                        # so each accumulation group owns its region cleanly
                        nc.tensor.matmul(
                            po[:, s, 0 : HD + 1],
                            at[:, g % RING, hh * QW + s * P : hh * QW + (s + 1) * P],
                            vt_t[:, kt, 2 * p + hh, :],
                            start=(kt == 0 and s == 0),
                            stop=(kt == ST - 1),
                            skip_group_check=True,
                        )
                if kt == ST - 1:
                    emit_norm(p, qc, tail=(p == 1 and qc == QC - 1))

            seq = [
                (qc, p, kt)
                for qc in range(QC)
                for p in range(2)
                for kt in range(ST)
            ]
            DELAY = 6
            # deferred per-group work, spread thin so PE per group stays
            # under the ACT exp rate (a qk half or proj sub is ~850ns of PE
            # vs ~200ns/group of slack): remaining q/k projection halves
            # land ~6 groups before their consumers; each qc's projection
            # sub-blocks trickle through the next qc's groups with the
            # ReduceScatter issued after the fourth.
            actions = {
                1: [("qk", 3, 2, 0)], 3: [("qk", 3, 2, 1)],   # K p1 qc2 by g24
                5: [("qk", 3, 3, 0)], 7: [("qk", 3, 3, 1)],   # K p1 qc3 by g28
                10: [("bias",)],
                22: [("qk", 0, 1, 0)], 24: [("qk", 0, 1, 1)],  # Q p0 qc1 by g32
                43: [("qk", 1, 1, 0)], 45: [("qk", 1, 1, 1)],  # Q p1 qc1 by g48
                54: [("qk", 0, 2, 0)], 56: [("qk", 0, 2, 1)],  # Q p0 qc2 by g64
                75: [("qk", 1, 2, 0)], 77: [("qk", 1, 2, 1)],  # Q p1 qc2 by g80
                86: [("qk", 0, 3, 0)], 88: [("qk", 0, 3, 1)],  # Q p0 qc3 by g96
                107: [("qk", 1, 3, 0)], 109: [("qk", 1, 3, 1)],  # Q p1 qc3 g112
            }
            # norm(1,qc) flushes at group 32qc+31+DELAY; projection sub-blocks
            # follow from +37, the ReduceScatter right after the fourth
            for qc in range(3):
                for g_, s in zip((40, 42, 44, 46), range(4)):
                    actions.setdefault(32 * qc + g_, []).append(("proj", qc, s))
                actions.setdefault(32 * qc + 46, []).append(("rs", qc))
            actions.setdefault(70, []).append(("outst", 0))
            actions.setdefault(102, []).append(("outst", 1))

            def run_action(a):
                if a[0] == "qk":
                    qk_half(*a[1:])
                elif a[0] == "proj":
                    proj_sub(a[1], a[2])
                elif a[0] == "rs":
                    emit_rs(a[1])
                elif a[0] == "outst":
                    emit_outst(a[1])
                elif a[0] == "bias":
                    # bias_bcast[p, n] = b_proj[n] (pre-scaled by 1/4 on host)
                    for j in range(2):
                        pb = pp.tile([P, QW], F32, name="pp")
                        nc.tensor.matmul(
                            pb[:], ones128[:], bpr_s[0:1, j * QW : (j + 1) * QW],
                            start=True, stop=True,
                        )
                        nc.vector.tensor_copy(
                            bias_bcast[:, j * QW : (j + 1) * QW], pb[:]
                        )

            proj_psum["tail_sp"] = sp
            for g, (qc, p, kt) in enumerate(seq):
                emit_scores(g, qc, p, kt)
                for a in actions.get(g, ()):
                    run_action(a)
                if g >= DELAY:
                    emit_av(g - DELAY, *seq[g - DELAY])
            for g in range(len(seq) - DELAY, len(seq)):
                emit_av(g, *seq[g])
            emit_rs(3)
            for qc in range(QC):
                emit_outst(qc)

    nc.compile()
    return nc


_CACHE = {}


def _get_nc(mode=MODE):
    if mode not in _CACHE:
        _CACHE[mode] = build(mode)
    return _CACHE[mode]


def make_in_maps(x, w_qkv, b_qkv, w_proj, b_proj):
    x = np.asarray(x, dtype=np.float32)
    w_qkv = np.asarray(w_qkv, dtype=np.float32)
    b_qkv = np.asarray(b_qkv, dtype=np.float32)
    w_proj = np.asarray(w_proj, dtype=np.float32)
    b_proj = np.asarray(b_proj, dtype=np.float32)
    in_maps = []
    for c in range(N_CORES):
        b, g = c // 4, c % 4
        f = slice(g * FQ, (g + 1) * FQ)
        fq = slice(g * FQ, (g + 1) * FQ)
        fk = slice(D + g * FQ, D + (g + 1) * FQ)
        fv = slice(2 * D + g * FQ, 2 * D + (g + 1) * FQ)
        in_maps.append(
            {
                "xt": np.ascontiguousarray(x[b].T),
                "wqk": np.ascontiguousarray(
                    np.concatenate([w_qkv[:, fq], w_qkv[:, fk]], axis=1)
                ),
                "wv": np.ascontiguousarray(w_qkv[:, fv]),
                "bqk": np.concatenate([b_qkv[fq], b_qkv[fk]]).reshape(2 * FQ, 1).copy(),
                "bv": b_qkv[fv].reshape(1, FQ).copy(),
                "wpr": np.ascontiguousarray(w_proj[f, :]),
                "bpr": (b_proj / 4.0).reshape(1, D).copy(),
            }
        )
    return in_maps


def assemble(results, mode=MODE):
    out = np.empty((B, S, D), dtype=np.float32)
    if mode in ("rs", "hybrid"):
        nrs = QC if mode == "rs" else QC - 1
        for c in range(N_CORES):
            b, i = c // 4, c % 4
            r = np.asarray(results[c]["out"], dtype=np.float32)  # [nrs, P, D]
            for qc in range(nrs):
                r0 = qc * QW + i * P
                out[b, r0 : r0 + P, :] = r[qc]
        if mode == "hybrid":
            t0 = (QC - 1) * QW
            for b in range(B):
                grp = [
                    np.asarray(results[4 * b + i]["pout"][t0:], dtype=np.float32)
                    for i in range(4)
                ]
                out[b, t0:] = grp[0] + grp[1] + grp[2] + grp[3]
    else:
        for b in range(B):
            grp = [
                np.asarray(results[4 * b + i]["out"], dtype=np.float32)
                for i in range(4)
            ]
            out[b] = grp[0] + grp[1] + grp[2] + grp[3]
    return out


def kernel(x, w_qkv, b_qkv, w_proj, b_proj, num_heads=H, **_):
    in_maps = make_in_maps(x, w_qkv, b_qkv, w_proj, b_proj)
    try:
        res = run_bass_kernel_spmd(
            _get_nc(MODE), in_maps, core_ids=list(range(N_CORES))
        )
        return assemble(res.results, MODE)
    except Exception:
        if MODE == "partial":
            raise
        # fallback: no-collective program, partial sums reduced on host
        res = run_bass_kernel_spmd(
            _get_nc("partial"), in_maps, core_ids=list(range(N_CORES))
        )
        return assemble(res.results, "partial")



# revision 31
# speedup vs baseline: 1.0452x; 1.0452x over previous
"""Fused multi-head attention forward for TRN2, SPMD over 8 NeuronCores.

Problem: B=2, S=2048, D=1024, H=16 heads (Hd=64), fp32.
  out = proj(softmax((x@Wq + bq)(x@Wk + bk)^T / 8) @ (x@Wv + bv))

Sharding: 2-way data parallel over batch x 4-way tensor parallel over heads.
Core c handles batch c//4 and heads [4*(c%4), 4*(c%4)+4). Attention is fully
local; the output projection is computed on each core over its 256 head
features (with bias/4) into a full [S, D] partial; the host sums the four
partials per batch during unshard.

v3 vs v2 (187.3 us -> target ~150 us):
- The v2 trace shows ACT (exp) busy 138 us but idle for the first 37 us
  because all of phase A (QKV+V projections) was emitted before the first
  score matmul; the span is ACT-bound after that. v3 removes phase A:
  a minimal prefix (Q pair0 chunk0 + K pair0 chunk0) starts the
  scores->exp pipeline within a few us of the first x stripe landing, and
  every remaining qk chunk / V half-tile / projection sub-block is placed
  by a deadline-driven latest-fit placer into per-group filler slots.
- V tiles split into per-pair halves (128 wv columns each) so pair-1
  halves inherit deadlines 16 groups later, easing the early PE crunch.
- DELAY raised 6 -> 12 (RING 14) to push V-tile deadlines later; the at
  ring grows to 28KB/partition which SBUF comfortably holds.
- Output stores issue on the Pool queue (SWDGE) instead of SP so the
  norm DMA-transposes never queue behind them on SP.SEQ.
"""
import sys

sys.path.insert(0, "/opt/trn_rl_repo")
from collections import defaultdict
from contextlib import ExitStack

import numpy as np

import concourse.bass as bass
import concourse.tile as tile
from concourse import bacc, mybir
from concourse.bass_utils import run_bass_kernel_spmd
from concourse.masks import make_identity

F32 = mybir.dt.float32
BF16 = mybir.dt.bfloat16
EXP = mybir.ActivationFunctionType.Exp
COPY = mybir.ActivationFunctionType.Copy

P = 128
B, S, D, H, HD = 2, 2048, 1024, 16, 64
NH = 4          # heads per core
FQ = NH * HD    # 256 q/k/v features per core
ST = S // P     # 16 seq tiles
KD = D // P     # 8 contraction tiles over d_model
QC = 4          # q chunks
QW = S // QC    # 512
N_CORES = 8
DELAY = 12      # groups between emit_scores(g) and emit_av(g)
RING = 14       # exp ring slots (> DELAY + 1)


def build():
    nc = bacc.Bacc(
        "TRN2",
        target_bir_lowering=False,
        debug=False,
        enable_asserts=False,
        num_devices=N_CORES,
    )
    xt_d = nc.dram_tensor("xt", [D, S], F32, kind="ExternalInput").ap()
    wqk_d = nc.dram_tensor("wqk", [D, 2 * FQ], F32, kind="ExternalInput").ap()
    wv_d = nc.dram_tensor("wv", [D, FQ], F32, kind="ExternalInput").ap()
    bqk_d = nc.dram_tensor("bqk", [2 * FQ, 1], F32, kind="ExternalInput").ap()
    bv_d = nc.dram_tensor("bv", [1, FQ], F32, kind="ExternalInput").ap()
    wpr_d = nc.dram_tensor("wpr", [FQ, D], F32, kind="ExternalInput").ap()
    bpr_d = nc.dram_tensor("bpr", [1, D], F32, kind="ExternalInput").ap()
    out_d = nc.dram_tensor("out", [S, D], BF16, kind="ExternalOutput").ap()

    with tile.TileContext(nc) as tc, ExitStack() as ctx:
        const = ctx.enter_context(tc.tile_pool(name="const", bufs=1))
        qkv = ctx.enter_context(tc.tile_pool(name="qkv", bufs=1))
        otp = ctx.enter_context(tc.tile_pool(name="otp", bufs=1))
        mis = ctx.enter_context(tc.tile_pool(name="mis", bufs=2))
        otqp = ctx.enter_context(tc.tile_pool(name="otqp", bufs=10))
        pp = ctx.enter_context(tc.tile_pool(name="pp", bufs=2, space="PSUM"))
        xa = ctx.enter_context(tc.tile_pool(name="xt", bufs=1))
        wa = ctx.enter_context(tc.tile_pool(name="wa", bufs=1))

        # ---- small bias DMAs first (SP/HWDGE, cheap, parallel to the Pool
        # SWDGE generation stream), needed at first drains
        bv_s = const.tile([1, FQ], F32)
        nc.sync.dma_start(bv_s[:], bv_d[:])
        bqk_s = []
        for m in range(4):
            t = const.tile([P, 1], F32, name=f"bqk{m}")
            nc.sync.dma_start(t[:], bqk_d[m * P : (m + 1) * P, :])
            bqk_s.append(t)

        # ---- inputs via batched SWDGE casting DMAs (f32 DRAM -> bf16 SBUF),
        # ordered so the prefix (Q pair0 chunk0, K pair0 chunk0) unblocks
        # first: Q-columns + x stripe 0 lead, K-columns next, then wv and the
        # remaining x stripes.
        xt3 = [xa.tile([P, KD, QW], BF16, name=f"x{c}") for c in range(QC)]
        wqk3 = wa.tile([P, KD, 2 * FQ], BF16, name="wqk")
        wv3 = wa.tile([P, KD, FQ], BF16, name="wv")
        xt_v = xt_d.rearrange("(k p) s -> p k s", p=P)
        wqk_v = wqk_d.rearrange("(k p) f -> p k f", p=P)

        # prefix consumes only wqk columns m0 (Q pair0, 0:P) and m2 (K pair0,
        # 2P:3P); bring exactly those first, finely interleaved with x0 so
        # the first score group unblocks as early as the DMA engines allow
        KH = KD // 2
        nc.gpsimd.dma_start(wqk3[:, 0:KH, 0:FQ], wqk_v[:, 0:KH, 0:FQ])
        nc.gpsimd.dma_start(xt3[0][:, 0:KH, :], xt_v[:, 0:KH, 0:QW])
        nc.gpsimd.dma_start(wqk3[:, KH:, 0:FQ], wqk_v[:, KH:, 0:FQ])
        nc.gpsimd.dma_start(xt3[0][:, KH:, :], xt_v[:, KH:, 0:QW])
        nc.gpsimd.dma_start(wqk3[:, :, FQ:], wqk_v[:, :, FQ:])
        nc.gpsimd.dma_start(wv3[:], wv_d.rearrange("(k p) f -> p k f", p=P))
        nc.gpsimd.dma_start(xt3[1][:], xt_v[:, :, QW : 2 * QW])
        nc.gpsimd.dma_start(xt3[2][:], xt_v[:, :, 2 * QW : 3 * QW])
        nc.gpsimd.dma_start(xt3[3][:], xt_v[:, :, 3 * QW : 4 * QW])
        wpr3 = wa.tile([P, 2, D], BF16, name="wpr")
        nc.gpsimd.dma_start(wpr3[:], wpr_d.rearrange("(j p) f -> p j f", p=P))
        bpr_s = const.tile([1, D], BF16)
        nc.gpsimd.dma_start(bpr_s[:], bpr_d[:])

        wqk_s = [wqk3[:, k, :] for k in range(KD)]
        wpr_s = [wpr3[:, j, :] for j in range(2)]

        # ---- small constants
        ones_f = const.tile([1, P], F32)
        nc.vector.memset(ones_f[:], 1.0)
        ones128 = const.tile([1, P], BF16)
        nc.vector.tensor_copy(ones128[:], ones_f[:])
        ident = const.tile([P, P], BF16, name="ident")
        make_identity(nc, ident[:])
        onesv = const.tile([P, ST, NH, 1], BF16)
        nc.vector.memset(onesv[:], 1.0)
        bias_bcast = const.tile([P, D], F32)

        qt_t = [qkv.tile([P, S], BF16, name=f"qt{i}") for i in range(2)]
        kt_t = [qkv.tile([P, S], BF16, name=f"kt{i}") for i in range(2)]
        vt_t = qkv.tile([P, ST, NH, HD + 1], BF16, name="vt")
        nc.vector.tensor_copy(vt_t[:, :, :, HD : HD + 1], onesv[:])

        # bv broadcast target (filled right after the prefix; each v_half
        # folds the bias into its PSUM-drain DVE op)
        bv_bcast = const.tile([P, FQ], F32, name="bvb")

        qk_pending = {}

        def qk_half(m, qc, half):
            # m-tile -> destination: 0,1 = Q pairs; 2,3 = K pairs
            if half == 0:
                qk_pending[(m, qc)] = pp.tile([P, QW], F32, name="pp")
            pq = qk_pending[(m, qc)]
            for k in range(half * KD // 2, (half + 1) * KD // 2):
                nc.tensor.matmul(
                    pq[:],
                    wqk_s[k][:, m * P : (m + 1) * P],
                    xt3[qc][:, k, :],
                    start=(k == 0),
                    stop=(k == KD - 1),
                )
            if half == 1:
                dest = qt_t[m] if m < 2 else kt_t[m - 2]
                nc.vector.tensor_scalar_add(
                    dest[:, qc * QW : (qc + 1) * QW], pq[:], bqk_s[m][:]
                )
                del qk_pending[(m, qc)]

        def v_half(st, pr):
            # V projection for sequence tile st, head pair pr (128 features)
            pv = pp.tile([P, FQ // 2], F32, name="pp")
            c, r = divmod(st * P, QW)
            cols = slice(pr * P, (pr + 1) * P)
            for k in range(KD):
                nc.tensor.matmul(
                    pv[:],
                    xt3[c][:, k, r : r + P],
                    wv3[:, k, cols],
                    start=(k == 0),
                    stop=(k == KD - 1),
                )
            nc.vector.tensor_add(
                vt_t[:, st, 2 * pr : 2 * pr + 2, 0:HD],
                pv[:].rearrange("p (a b) -> p a b", a=2),
                bv_bcast[:, cols].rearrange("p (a b) -> p a b", a=2),
            )

        # ---- attention pipeline state
        ot_t = [otp.tile([P, S], BF16, name=f"ot{i}") for i in range(2)]

        proj_out = {}

        def proj_half(qc, sub, j):
            # one j-half of a projection sub-block: 2 matmuls + DVE bias-add
            qt = qc * 4 + sub
            ts = slice(qt * P, (qt + 1) * P)
            if j == 0:
                proj_out[(qc, sub)] = mis.tile([P, D], BF16, name="outsb")
            outsb = proj_out[(qc, sub)]
            js = slice(j * QW, (j + 1) * QW)
            ppp = pp.tile([P, QW], F32, name="pp")
            nc.tensor.matmul(
                ppp[:], ot_t[0][:, ts], wpr_s[0][:, js], start=True, stop=False
            )
            nc.tensor.matmul(
                ppp[:], ot_t[1][:, ts], wpr_s[1][:, js], start=False, stop=True
            )
            nc.vector.tensor_add(outsb[:, js], ppp[:], bias_bcast[:, js])
            if j == 1:
                nc.gpsimd.dma_start(out_d[ts, :], outsb[:])
                del proj_out[(qc, sub)]

        def proj_sub(qc, sub, tail=False):
            qt = qc * 4 + sub
            ts = slice(qt * P, (qt + 1) * P)
            outsb = mis.tile([P, D], BF16, name="outsb")
            if tail:
                # post-last-exp: DVE is busy with the norm muls/drains, ACT
                # is idle -> fold bias on the PE, drain with one ACT copy
                pt = sp.tile([P, 2 * QW], F32, name="ps")
                for j in range(2):
                    js = slice(j * QW, (j + 1) * QW)
                    nc.tensor.matmul(
                        pt[:, js], ot_t[0][:, ts], wpr_s[0][:, js],
                        start=True, stop=False,
                    )
                    nc.tensor.matmul(
                        pt[:, js], ot_t[1][:, ts], wpr_s[1][:, js],
                        start=False, stop=False,
                    )
                    nc.tensor.matmul(
                        pt[:, js], ones128[:], bpr_s[0:1, js],
                        start=False, stop=True,
                    )
                nc.scalar.activation(outsb[:], pt[:], COPY, bias=0.0, scale=1.0)
                nc.sync.dma_start(out_d[ts, :], outsb[:])
                return
            for j in range(2):
                js = slice(j * QW, (j + 1) * QW)
                ppp = pp.tile([P, QW], F32, name="pp")
                nc.tensor.matmul(
                    ppp[:], ot_t[0][:, ts], wpr_s[0][:, js],
                    start=True, stop=False,
                )
                nc.tensor.matmul(
                    ppp[:], ot_t[1][:, ts], wpr_s[1][:, js],
                    start=False, stop=True,
                )
                nc.vector.tensor_add(outsb[:, js], ppp[:], bias_bcast[:, js])
            nc.gpsimd.dma_start(out_d[ts, :], outsb[:])

        def bias_fn():
            # bias_bcast[p, n] = b_proj[n] (pre-scaled by 1/4 on host)
            for j in range(2):
                pb = pp.tile([P, QW], F32, name="pp")
                nc.tensor.matmul(
                    pb[:], ones128[:], bpr_s[0:1, j * QW : (j + 1) * QW],
                    start=True, stop=True,
                )
                nc.vector.tensor_copy(bias_bcast[:, j * QW : (j + 1) * QW], pb[:])

        with ExitStack() as ctx_b:
            att = ctx_b.enter_context(tc.tile_pool(name="att", bufs=1))
            sp = ctx_b.enter_context(tc.tile_pool(name="sp", bufs=2, space="PSUM"))
            op = ctx_b.enter_context(tc.tile_pool(name="op", bufs=1, space="PSUM"))

            at = att.tile([P, RING, 2 * QW], BF16, name="at")
            po_cur = {}

            def emit_scores(g, qc, p, kt):
                qs = slice(qc * QW, (qc + 1) * QW)
                ks = slice(kt * P, (kt + 1) * P)
                ps = sp.tile([P, 2 * QW], F32, name="ps")
                nc.tensor.matmul(
                    ps[:, 0:QW], kt_t[p][0:64, ks], qt_t[p][0:64, qs],
                    start=True, stop=True, tile_position=(0, 0),
                )
                nc.tensor.matmul(
                    ps[:, QW : 2 * QW], kt_t[p][64:128, ks], qt_t[p][64:128, qs],
                    start=True, stop=True, tile_position=(64, 0),
                )
                nc.scalar.activation(
                    at[:, g % RING, :], ps[:], EXP, bias=0.0, scale=0.125
                )

            def norm_sub(p, qc, po0, po1, recips, s, pe_transpose=False):
                ts = slice(qc * QW + s * P, qc * QW + (s + 1) * P)
                otq = otqp.tile([P, 2, HD], BF16, name="otq")
                nc.vector.tensor_scalar_mul(
                    otq[:, 0, :], po0[:, s, 0:HD], recips[:, 0, s, :]
                )
                nc.vector.tensor_scalar_mul(
                    otq[:, 1, :], po1[:, s, 0:HD], recips[:, 1, s, :]
                )
                if pe_transpose:
                    # tail path: the DMA-transpose (serial HWDGE gen + 0.9us
                    # DMA-completion semaphore) is too slow on the critical
                    # tail; transpose on the PE and drain via idle DVE
                    tp = sp.tile([P, P], BF16, name="ps")
                    nc.tensor.transpose(tp[0:64, :], otq[:, 0, :], ident[:])
                    nc.tensor.transpose(tp[64:128, :], otq[:, 1, :], ident[:])
                    nc.vector.tensor_copy(ot_t[p][:, ts], tp[:])
                else:
                    nc.sync.dma_start(ot_t[p][:, ts], otq[:], transpose=True)

            def emit_norm(p, qc, tail=False):
                po0, po1 = po_cur.pop((p, qc))
                recips = otqp.tile([P, 2, NH, 1], F32, name="recips")
                with nc.allow_low_precision(reason="softmax recip"):
                    nc.vector.reciprocal(recips[:, 0, :, :], po0[:, :, HD : HD + 1])
                    nc.vector.reciprocal(recips[:, 1, :, :], po1[:, :, HD : HD + 1])
                for s in range(4):
                    norm_sub(p, qc, po0, po1, recips, s, pe_transpose=tail)
                    if tail:
                        proj_sub(qc, s, tail=True)

            def emit_av(g, qc, p, kt):
                if kt == 0:
                    po_cur[(p, qc)] = (
                        op.tile([P, NH, P], F32, name="po0"),
                        op.tile([P, NH, P], F32, name="po1"),
                    )
                po0, po1 = po_cur[(p, qc)]
                for hh, po in ((0, po0), (1, po1)):
                    for s in range(4):
                        # sub-regions padded to the 512B PSUM zero-region
                        # so each accumulation group owns its region cleanly
                        nc.tensor.matmul(
                            po[:, s, 0 : HD + 1],
                            at[:, g % RING, hh * QW + s * P : hh * QW + (s + 1) * P],
                            vt_t[:, kt, 2 * p + hh, :],
                            start=(kt == 0 and s == 0),
                            stop=(kt == ST - 1),
                            skip_group_check=True,
                        )
                if kt == ST - 1:
                    emit_norm(p, qc, tail=(p == 1 and qc == QC - 1))

            seq = [
                (qc, p, kt)
                for qc in range(QC)
                for p in range(2)
                for kt in range(ST)
            ]

            # ---- deadline-driven filler schedule -------------------------
            # Each unit: (latest_emit_group, est_PE_us, chain, fn). Latest-fit
            # into per-group budgets; overflow spills into the prefix. Units
            # sharing a chain are re-bound to their assigned slots in order,
            # so a chunk's half-1 never executes before its half-0.
            units = []

            def qkh(m, qc, h):
                return lambda: qk_half(m, qc, h)

            def vh(st, pr):
                return lambda: v_half(st, pr)

            def prj(qc, s_):
                return lambda: proj_sub(qc, s_)

            def add_qk(m, qc, e):
                # -2 margin: the chunk's DVE bias-add drain + semaphore chain
                # land ~1 group after the PE half finishes
                units.append([e - 3, 0.85, (m, qc), qkh(m, qc, 0)])
                units.append([e - 2, 0.85, (m, qc), qkh(m, qc, 1)])

            # K pair0 chunks 1-3 (chunk c first used by scores group 4c)
            for c in (1, 2, 3):
                add_qk(2, c, 4 * c - 1)
            # Q pair1 qc0 + K pair1 chunks (first used at group 16 + 4c)
            add_qk(1, 0, 15)
            for c in range(4):
                add_qk(3, c, 16 + 4 * c - 1)
            # Q chunks 1-3 for both pairs (first used at 32qc / 32qc+16)
            for qc in (1, 2, 3):
                add_qk(0, qc, 32 * qc - 1)
                add_qk(1, qc, 32 * qc + 15)
            # V halves: pair0 feeds av(qc0,p0,st) at group st+DELAY; pair1
            # feeds av(qc0,p1,st) at group 16+st+DELAY (capped before projs)
            for st in range(ST):
                units.append([st + DELAY - 2, 0.43, None, vh(st, 0)])
                units.append(
                    [min(st + 16 + DELAY - 2, 40), 0.43, None, vh(st, 1)]
                )
            units.append([34, 0.45, None, bias_fn])

            NG = len(seq)
            budget = [0.50 if g < 28 else 0.40 for g in range(NG)]
            sched = defaultdict(list)
            # fixed-position projection sub-blocks: norm(1,qc) is emitted at
            # loop group 32qc+31+DELAY; spread the 4 subs right after, and
            # pre-charge their PE cost so the placer avoids those groups
            for qc in range(3):
                for s_ in range(4):
                    g_ = 32 * qc + 32 + DELAY + 2 * s_
                    sched[g_].append(prj(qc, s_))
                    budget[g_] -= 1.05
            placed = []  # (group or -1 for prefix, order, chain, fn)
            for e, cost, chain, fn in sorted(units, key=lambda u: u[0]):
                g = min(e, NG - 1)
                while g >= 0 and budget[g] <= 1e-9:
                    g -= 1
                if g >= 0:
                    budget[g] -= cost
                placed.append([g, chain, fn])
            # re-bind chained units: sort each chain's slots, keep fn order
            by_chain = defaultdict(list)
            for i, (g, chain, fn) in enumerate(placed):
                if chain is not None:
                    by_chain[chain].append(i)
            for idxs in by_chain.values():
                slots = sorted(placed[i][0] for i in idxs)
                for i, s_ in zip(idxs, slots):
                    placed[i][0] = s_
            prefix_units = []
            for g, chain, fn in placed:
                if g < 0:
                    prefix_units.append(fn)
                else:
                    sched[g].append(fn)

            # ---- PE warm-up: junk matmuls on constants while the first x/w
            # DMAs are in flight, so the p-state ramp (0.65 -> 2.4 GHz over a
            # 3us busy streak) completes before the real prefix work starts
            junk = sp.tile([P, 2 * QW], F32, name="ps")
            for _ in range(26):
                nc.tensor.matmul(
                    junk[:, 0:P], ones128[:], ones128[:], start=True, stop=True
                )

            # ---- prefix: just enough to start the pipeline
            qk_half(0, 0, 0)
            qk_half(0, 0, 1)   # Q pair0 chunk0
            qk_half(2, 0, 0)
            qk_half(2, 0, 1)   # K pair0 chunk0
            pb0 = pp.tile([P, FQ], F32, name="pp")
            nc.tensor.matmul(pb0[:], ones_f[:], bv_s[:], start=True, stop=True)
            nc.vector.tensor_copy(bv_bcast[:], pb0[:])
            for fn in prefix_units:
                fn()

            for g, (qc, p, kt) in enumerate(seq):
                # av first (its exp finished long before this group's scores
                # dep) -- EXCEPT a chunk's first av, which waits on the
                # previous chunk's norm to free the po region and would clog
                # the PE wait-queue ahead of the scores
                av_first = g >= DELAY and seq[g - DELAY][2] != 0
                if av_first:
                    emit_av(g - DELAY, *seq[g - DELAY])
                emit_scores(g, qc, p, kt)
                for fn in sched.get(g, ()):
                    fn()
                if g >= DELAY and not av_first:
                    emit_av(g - DELAY, *seq[g - DELAY])
            for g in range(NG - DELAY, NG):
                emit_av(g, *seq[g])

    nc.compile()
    return nc


_CACHE = {}


def _get_nc():
    if "nc" not in _CACHE:
        _CACHE["nc"] = build()
    return _CACHE["nc"]


def make_in_maps(x, w_qkv, b_qkv, w_proj, b_proj):
    x = np.asarray(x, dtype=np.float32)
    w_qkv = np.asarray(w_qkv, dtype=np.float32)
    b_qkv = np.asarray(b_qkv, dtype=np.float32)
    w_proj = np.asarray(w_proj, dtype=np.float32)
    b_proj = np.asarray(b_proj, dtype=np.float32)
    in_maps = []
    for c in range(N_CORES):
        b, g = c // 4, c % 4
        f = slice(g * FQ, (g + 1) * FQ)
        fq = slice(g * FQ, (g + 1) * FQ)
        fk = slice(D + g * FQ, D + (g + 1) * FQ)
        fv = slice(2 * D + g * FQ, 2 * D + (g + 1) * FQ)
        in_maps.append(
            {
                "xt": np.ascontiguousarray(x[b].T),
                "wqk": np.ascontiguousarray(
                    np.concatenate([w_qkv[:, fq], w_qkv[:, fk]], axis=1)
                ),
                "wv": np.ascontiguousarray(w_qkv[:, fv]),
                "bqk": np.concatenate([b_qkv[fq], b_qkv[fk]]).reshape(2 * FQ, 1).copy(),
                "bv": b_qkv[fv].reshape(1, FQ).copy(),
                "wpr": np.ascontiguousarray(w_proj[f, :]),
                "bpr": (b_proj / 4.0).reshape(1, D).copy(),
            }
        )
    return in_maps


def assemble(results):
    out = np.empty((B, S, D), dtype=np.float32)
    for b in range(B):
        grp = [
            np.asarray(results[4 * b + i]["out"], dtype=np.float32)
            for i in range(4)
        ]
        out[b] = grp[0] + grp[1] + grp[2] + grp[3]
    return out


def kernel(x, w_qkv, b_qkv, w_proj, b_proj, num_heads=H, **_):
    in_maps = make_in_maps(x, w_qkv, b_qkv, w_proj, b_proj)
    res = run_bass_kernel_spmd(
        _get_nc(), in_maps, core_ids=list(range(N_CORES))
    )
    return assemble(res.results)


# revision 52
# speedup vs baseline: 1.1283x; 1.0795x over previous
"""Fused multi-head attention forward for TRN2, SPMD over 8 NeuronCores.

Problem: B=2, S=2048, D=1024, H=16 heads (Hd=64), fp32.
  out = proj(softmax((x@Wq + bq)(x@Wk + bk)^T / 8) @ (x@Wv + bv))

Sharding: 2-way data parallel over batch x 4-way tensor parallel over heads.
Core c handles batch c//4 and heads [4*(c%4), 4*(c%4)+4). Attention is fully
local; the output projection is computed on each core over its 256 head
features (with bias/4) into a full [S, D] partial; the host sums the four
partials per batch during unshard.

v3 vs v2 (187.3 us -> target ~150 us):
- The v2 trace shows ACT (exp) busy 138 us but idle for the first 37 us
  because all of phase A (QKV+V projections) was emitted before the first
  score matmul; the span is ACT-bound after that. v3 removes phase A:
  a minimal prefix (Q pair0 chunk0 + K pair0 chunk0) starts the
  scores->exp pipeline within a few us of the first x stripe landing, and
  every remaining qk chunk / V half-tile / projection sub-block is placed
  by a deadline-driven latest-fit placer into per-group filler slots.
- V tiles split into per-pair halves (128 wv columns each) so pair-1
  halves inherit deadlines 16 groups later, easing the early PE crunch.
- DELAY raised 6 -> 12 (RING 14) to push V-tile deadlines later; the at
  ring grows to 28KB/partition which SBUF comfortably holds.
- Output stores issue on the Pool queue (SWDGE) instead of SP so the
  norm DMA-transposes never queue behind them on SP.SEQ.
"""
import os
import sys

sys.path.insert(0, "/opt/trn_rl_repo")
from collections import defaultdict
from contextlib import ExitStack

import numpy as np

import concourse.bass as bass
import concourse.tile as tile
from concourse import bacc, mybir
from concourse.bass_utils import run_bass_kernel_spmd
from concourse.masks import make_identity

F32 = mybir.dt.float32
BF16 = mybir.dt.bfloat16
EXP = mybir.ActivationFunctionType.Exp
COPY = mybir.ActivationFunctionType.Copy

P = 128
B, S, D, H, HD = 2, 2048, 1024, 16, 64
NH = 4          # heads per core
FQ = NH * HD    # 256 q/k/v features per core
ST = S // P     # 16 seq tiles
KD = D // P     # 8 contraction tiles over d_model
QC = 4          # q chunks
QW = S // QC    # 512
N_CORES = 8
DELAY = int(os.environ.get("MHA_DELAY", "38"))  # emit_scores(g) -> emit_av(g)
RING = DELAY + 2                                # exp ring slots
B_EARLY = float(os.environ.get("MHA_BE", "0.50"))
B_STEADY = float(os.environ.get("MHA_BS", "0.40"))
VCAP = int(os.environ.get("MHA_VCAP", "99"))


def build():
    nc = bacc.Bacc(
        "TRN2",
        target_bir_lowering=False,
        debug=False,
        enable_asserts=False,
        num_devices=N_CORES,
    )
    xt_d = nc.dram_tensor("xt", [D, S], F32, kind="ExternalInput").ap()
    wqk_d = nc.dram_tensor("wqk", [D, 2 * FQ], F32, kind="ExternalInput").ap()
    wv_d = nc.dram_tensor("wv", [D, FQ], F32, kind="ExternalInput").ap()
    bqk_d = nc.dram_tensor("bqk", [2 * FQ, 1], F32, kind="ExternalInput").ap()
    bv_d = nc.dram_tensor("bv", [1, FQ], F32, kind="ExternalInput").ap()
    wpr_d = nc.dram_tensor("wpr", [FQ, D], F32, kind="ExternalInput").ap()
    bpr_d = nc.dram_tensor("bpr", [1, D], F32, kind="ExternalInput").ap()
    out_d = nc.dram_tensor("out", [S, D], BF16, kind="ExternalOutput").ap()

    with tile.TileContext(nc) as tc, ExitStack() as ctx:
        const = ctx.enter_context(tc.tile_pool(name="const", bufs=1))
        qkv = ctx.enter_context(tc.tile_pool(name="qkv", bufs=1))
        otp = ctx.enter_context(tc.tile_pool(name="otp", bufs=1))
        mis = ctx.enter_context(tc.tile_pool(name="mis", bufs=2))
        otqp = ctx.enter_context(tc.tile_pool(name="otqp", bufs=10))
        pp = ctx.enter_context(tc.tile_pool(name="pp", bufs=2, space="PSUM"))
        xa = ctx.enter_context(tc.tile_pool(name="xt", bufs=1))
        wa = ctx.enter_context(tc.tile_pool(name="wa", bufs=1))

        # ---- small bias DMAs first (SP/HWDGE, cheap, parallel to the Pool
        # SWDGE generation stream), needed at first drains
        bv_s = const.tile([1, FQ], F32)
        nc.sync.dma_start(bv_s[:], bv_d[:])
        bqk_s = []
        for m in range(4):
            t = const.tile([P, 1], F32, name=f"bqk{m}")
            nc.sync.dma_start(t[:], bqk_d[m * P : (m + 1) * P, :])
            bqk_s.append(t)

        # ---- inputs via batched SWDGE casting DMAs (f32 DRAM -> bf16 SBUF),
        # ordered so the prefix (Q pair0 chunk0, K pair0 chunk0) unblocks
        # first: Q-columns + x stripe 0 lead, K-columns next, then wv and the
        # remaining x stripes.
        xt3 = [xa.tile([P, KD, QW], BF16, name=f"x{c}") for c in range(QC)]
        wqk3 = wa.tile([P, KD, 2 * FQ], BF16, name="wqk")
        wv3 = wa.tile([P, KD, FQ], BF16, name="wv")
        xt_v = xt_d.rearrange("(k p) s -> p k s", p=P)
        wqk_v = wqk_d.rearrange("(k p) f -> p k f", p=P)

        # prefix consumes only wqk columns m0 (Q pair0, 0:P) and m2 (K pair0,
        # 2P:3P); bring exactly those first, finely interleaved with x0 so
        # the first score group unblocks as early as the DMA engines allow
        KH = KD // 2
        nc.gpsimd.dma_start(wqk3[:, 0:KH, 0:FQ], wqk_v[:, 0:KH, 0:FQ])
        nc.gpsimd.dma_start(xt3[0][:, 0:KH, :], xt_v[:, 0:KH, 0:QW])
        nc.gpsimd.dma_start(wqk3[:, KH:, 0:FQ], wqk_v[:, KH:, 0:FQ])
        nc.gpsimd.dma_start(xt3[0][:, KH:, :], xt_v[:, KH:, 0:QW])
        nc.gpsimd.dma_start(wqk3[:, :, FQ:], wqk_v[:, :, FQ:])
        nc.gpsimd.dma_start(wv3[:], wv_d.rearrange("(k p) f -> p k f", p=P))
        nc.gpsimd.dma_start(xt3[1][:], xt_v[:, :, QW : 2 * QW])
        nc.gpsimd.dma_start(xt3[2][:], xt_v[:, :, 2 * QW : 3 * QW])
        nc.gpsimd.dma_start(xt3[3][:], xt_v[:, :, 3 * QW : 4 * QW])
        wpr3 = wa.tile([P, 2, D], BF16, name="wpr")
        nc.gpsimd.dma_start(wpr3[:], wpr_d.rearrange("(j p) f -> p j f", p=P))
        bpr_s = const.tile([1, D], BF16)
        nc.gpsimd.dma_start(bpr_s[:], bpr_d[:])

        wqk_s = [wqk3[:, k, :] for k in range(KD)]
        wpr_s = [wpr3[:, j, :] for j in range(2)]

        # ---- small constants
        ones_f = const.tile([1, P], F32)
        nc.vector.memset(ones_f[:], 1.0)
        ones128 = const.tile([1, P], BF16)
        nc.vector.tensor_copy(ones128[:], ones_f[:])
        ident = const.tile([P, P], BF16, name="ident")
        make_identity(nc, ident[:])
        onesv = const.tile([P, ST, NH, 1], BF16)
        nc.vector.memset(onesv[:], 1.0)
        bias_bcast = const.tile([P, D], F32)

        qt_t = [qkv.tile([P, S], BF16, name=f"qt{i}") for i in range(2)]
        kt_t = [qkv.tile([P, S], BF16, name=f"kt{i}") for i in range(2)]
        vt_t = qkv.tile([P, ST, NH, HD + 1], BF16, name="vt")
        nc.vector.tensor_copy(vt_t[:, :, :, HD : HD + 1], onesv[:])

        # bv broadcast target (filled right after the prefix; each v_half
        # folds the bias into its PSUM-drain DVE op)
        bv_bcast = const.tile([P, FQ], F32, name="bvb")

        qk_pending = {}

        def qk_half(m, qc, half):
            # m-tile -> destination: 0,1 = Q pairs; 2,3 = K pairs
            if half == 0:
                qk_pending[(m, qc)] = pp.tile([P, QW], F32, name="pp")
            pq = qk_pending[(m, qc)]
            for k in range(half * KD // 2, (half + 1) * KD // 2):
                nc.tensor.matmul(
                    pq[:],
                    wqk_s[k][:, m * P : (m + 1) * P],
                    xt3[qc][:, k, :],
                    start=(k == 0),
                    stop=(k == KD - 1),
                )
            if half == 1:
                dest = qt_t[m] if m < 2 else kt_t[m - 2]
                nc.vector.tensor_scalar_add(
                    dest[:, qc * QW : (qc + 1) * QW], pq[:], bqk_s[m][:]
                )
                del qk_pending[(m, qc)]

        def v_half(st, pr):
            # V projection for sequence tile st, head pair pr (128 features)
            pv = pp.tile([P, FQ // 2], F32, name="pp")
            c, r = divmod(st * P, QW)
            cols = slice(pr * P, (pr + 1) * P)
            for k in range(KD):
                nc.tensor.matmul(
                    pv[:],
                    xt3[c][:, k, r : r + P],
                    wv3[:, k, cols],
                    start=(k == 0),
                    stop=(k == KD - 1),
                )
            nc.vector.tensor_add(
                vt_t[:, st, 2 * pr : 2 * pr + 2, 0:HD],
                pv[:].rearrange("p (a b) -> p a b", a=2),
                bv_bcast[:, cols].rearrange("p (a b) -> p a b", a=2),
            )

        # ---- attention pipeline state
        ot_t = [otp.tile([P, S], BF16, name=f"ot{i}") for i in range(2)]

        proj_out = {}

        def proj_half(qc, sub, j):
            # one j-half of a projection sub-block: 2 matmuls + DVE bias-add
            qt = qc * 4 + sub
            ts = slice(qt * P, (qt + 1) * P)
            if j == 0:
                proj_out[(qc, sub)] = mis.tile([P, D], BF16, name="outsb")
            outsb = proj_out[(qc, sub)]
            js = slice(j * QW, (j + 1) * QW)
            ppp = pp.tile([P, QW], F32, name="pp")
            nc.tensor.matmul(
                ppp[:], ot_t[0][:, ts], wpr_s[0][:, js], start=True, stop=False
            )
            nc.tensor.matmul(
                ppp[:], ot_t[1][:, ts], wpr_s[1][:, js], start=False, stop=True
            )
            nc.vector.tensor_add(outsb[:, js], ppp[:], bias_bcast[:, js])
            if j == 1:
                nc.gpsimd.dma_start(out_d[ts, :], outsb[:])
                del proj_out[(qc, sub)]

        def proj_sub(qc, sub, tail=False):
            qt = qc * 4 + sub
            ts = slice(qt * P, (qt + 1) * P)
            outsb = mis.tile([P, D], BF16, name="outsb")
            if tail:
                # post-last-exp: DVE is busy with the norm muls/drains, ACT
                # is idle -> fold bias on the PE, drain with one ACT copy
                pt = sp.tile([P, 2 * QW], F32, name="ps")
                for j in range(2):
                    js = slice(j * QW, (j + 1) * QW)
                    nc.tensor.matmul(
                        pt[:, js], ot_t[0][:, ts], wpr_s[0][:, js],
                        start=True, stop=False,
                    )
                    nc.tensor.matmul(
                        pt[:, js], ot_t[1][:, ts], wpr_s[1][:, js],
                        start=False, stop=False,
                    )
                    nc.tensor.matmul(
                        pt[:, js], ones128[:], bpr_s[0:1, js],
                        start=False, stop=True,
                    )
                nc.scalar.activation(outsb[:], pt[:], COPY, bias=0.0, scale=1.0)
                nc.sync.dma_start(out_d[ts, :], outsb[:])
                return
            for j in range(2):
                js = slice(j * QW, (j + 1) * QW)
                ppp = pp.tile([P, QW], F32, name="pp")
                nc.tensor.matmul(
                    ppp[:], ot_t[0][:, ts], wpr_s[0][:, js],
                    start=True, stop=False,
                )
                nc.tensor.matmul(
                    ppp[:], ot_t[1][:, ts], wpr_s[1][:, js],
                    start=False, stop=True,
                )
                nc.vector.tensor_add(outsb[:, js], ppp[:], bias_bcast[:, js])
            nc.gpsimd.dma_start(out_d[ts, :], outsb[:])

        def bias_fn():
            # bias_bcast[p, n] = b_proj[n] (pre-scaled by 1/4 on host)
            for j in range(2):
                pb = pp.tile([P, QW], F32, name="pp")
                nc.tensor.matmul(
                    pb[:], ones128[:], bpr_s[0:1, j * QW : (j + 1) * QW],
                    start=True, stop=True,
                )
                nc.vector.tensor_copy(bias_bcast[:, j * QW : (j + 1) * QW], pb[:])

        with ExitStack() as ctx_b:
            att = ctx_b.enter_context(tc.tile_pool(name="att", bufs=1))
            sp = ctx_b.enter_context(tc.tile_pool(name="sp", bufs=2, space="PSUM"))
            op = ctx_b.enter_context(tc.tile_pool(name="op", bufs=1, space="PSUM"))

            at = att.tile([P, RING, 2 * QW], BF16, name="at")
            po_cur = {}

            def emit_scores(g, qc, p, kt):
                qs = slice(qc * QW, (qc + 1) * QW)
                ks = slice(kt * P, (kt + 1) * P)
                ps = sp.tile([P, 2 * QW], F32, name="ps")
                nc.tensor.matmul(
                    ps[:, 0:QW], kt_t[p][0:64, ks], qt_t[p][0:64, qs],
                    start=True, stop=True, tile_position=(0, 0),
                )
                nc.tensor.matmul(
                    ps[:, QW : 2 * QW], kt_t[p][64:128, ks], qt_t[p][64:128, qs],
                    start=True, stop=True, tile_position=(64, 0),
                )
                nc.scalar.activation(
                    at[:, g % RING, :], ps[:], EXP, bias=0.0, scale=0.125
                )

            def norm_sub(p, qc, po0, po1, recips, s, pe_transpose=False):
                ts = slice(qc * QW + s * P, qc * QW + (s + 1) * P)
                otq = otqp.tile([P, 2, HD], BF16, name="otq")
                nc.vector.tensor_scalar_mul(
                    otq[:, 0, :], po0[:, s, 0:HD], recips[:, 0, s, :]
                )
                nc.vector.tensor_scalar_mul(
                    otq[:, 1, :], po1[:, s, 0:HD], recips[:, 1, s, :]
                )
                if pe_transpose:
                    # tail path: the DMA-transpose (serial HWDGE gen + 0.9us
                    # DMA-completion semaphore) is too slow on the critical
                    # tail; transpose on the PE and drain via idle DVE
                    tp = sp.tile([P, P], BF16, name="ps")
                    nc.tensor.transpose(tp[0:64, :], otq[:, 0, :], ident[:])
                    nc.tensor.transpose(tp[64:128, :], otq[:, 1, :], ident[:])
                    nc.vector.tensor_copy(ot_t[p][:, ts], tp[:])
                else:
                    nc.sync.dma_start(ot_t[p][:, ts], otq[:], transpose=True)

            def emit_norm(p, qc, tail=False):
                po0, po1 = po_cur.pop((p, qc))
                recips = otqp.tile([P, 2, NH, 1], F32, name="recips")
                with nc.allow_low_precision(reason="softmax recip"):
                    nc.vector.reciprocal(recips[:, 0, :, :], po0[:, :, HD : HD + 1])
                    nc.vector.reciprocal(recips[:, 1, :, :], po1[:, :, HD : HD + 1])
                if tail:
                    for s in range(4):
                        norm_sub(p, qc, po0, po1, recips, s, pe_transpose=True)
                        proj_sub(qc, s, tail=True)
                    return
                # hh-major mul order: po0's region is fully read after 4 muls
                # (not 7), so the next chunk's first av unblocks ~0.6us sooner
                otqs = [otqp.tile([P, 2, HD], BF16, name="otq") for _ in range(4)]
                for s in range(4):
                    nc.vector.tensor_scalar_mul(
                        otqs[s][:, 0, :], po0[:, s, 0:HD], recips[:, 0, s, :]
                    )
                for s in range(4):
                    nc.vector.tensor_scalar_mul(
                        otqs[s][:, 1, :], po1[:, s, 0:HD], recips[:, 1, s, :]
                    )
                for s in range(4):
                    ts = slice(qc * QW + s * P, qc * QW + (s + 1) * P)
                    nc.sync.dma_start(ot_t[p][:, ts], otqs[s][:], transpose=True)

            def emit_av(g, qc, p, kt):
                if kt == 0:
                    po_cur[(p, qc)] = (
                        op.tile([P, NH, P], F32, name="po0"),
                        op.tile([P, NH, P], F32, name="po1"),
                    )
                po0, po1 = po_cur[(p, qc)]
                for hh, po in ((0, po0), (1, po1)):
                    for s in range(4):
                        # sub-regions padded to the 512B PSUM zero-region
                        # so each accumulation group owns its region cleanly
                        nc.tensor.matmul(
                            po[:, s, 0 : HD + 1],
                            at[:, g % RING, hh * QW + s * P : hh * QW + (s + 1) * P],
                            vt_t[:, kt, 2 * p + hh, :],
                            start=(kt == 0 and s == 0),
                            stop=(kt == ST - 1),
                            skip_group_check=True,
                        )
                if kt == ST - 1:
                    emit_norm(p, qc, tail=(p == 1 and qc == QC - 1))

            seq = [
                (qc, p, kt)
                for qc in range(QC)
                for p in range(2)
                for kt in range(ST)
            ]

            # ---- deadline-driven filler schedule -------------------------
            # Each unit: (latest_emit_group, est_PE_us, chain, fn). Latest-fit
            # into per-group budgets; overflow spills into the prefix. Units
            # sharing a chain are re-bound to their assigned slots in order,
            # so a chunk's half-1 never executes before its half-0.
            units = []

            def qkh(m, qc, h):
                return lambda: qk_half(m, qc, h)

            def vh(st, pr):
                return lambda: v_half(st, pr)

            def prj(qc, s_, j_):
                return lambda: proj_half(qc, s_, j_)

            def add_qk(m, qc, e):
                # -2 margin: the chunk's DVE bias-add drain + semaphore chain
                # land ~1 group after the PE half finishes
                units.append([e - 3, 0.85, (m, qc), qkh(m, qc, 0)])
                units.append([e - 2, 0.85, (m, qc), qkh(m, qc, 1)])

            # K pair0 chunks 1-3 (chunk c first used by scores group 4c)
            for c in (1, 2, 3):
                add_qk(2, c, 4 * c - 1)
            # Q pair1 qc0 + K pair1 chunks (first used at group 16 + 4c)
            add_qk(1, 0, 15)
            for c in range(4):
                add_qk(3, c, 16 + 4 * c - 1)
            # Q chunks 1-3 for both pairs (first used at 32qc / 32qc+16)
            for qc in (1, 2, 3):
                add_qk(0, qc, 32 * qc - 1)
                add_qk(1, qc, 32 * qc + 15)
            # V halves: pair0 feeds av(qc0,p0,st) at group st+DELAY; pair1
            # feeds av(qc0,p1,st) at group 16+st+DELAY (capped before projs)
            for st in range(ST):
                units.append([st + DELAY - 2, 0.43, None, vh(st, 0)])
                units.append(
                    [min(st + 16 + DELAY - 2, VCAP), 0.43, None, vh(st, 1)]
                )
            units.append([34, 0.45, None, bias_fn])

            NG = len(seq)
            budget = [B_EARLY if g < 28 else B_STEADY for g in range(NG)]
            sched = defaultdict(list)
            # fixed-position projection j-halves: norm(1,qc) is emitted at
            # loop group 32qc+31+DELAY; spread the 8 halves right after, and
            # pre-charge their PE cost so the placer avoids those groups
            # av emission trails scores by DELAY, then catches up two-per-group
            # over the last DELAY iterations so no avs (and no projections)
            # are left serialized after the final score group
            def av_due(g):
                return g - DELAY

            post_loop = defaultdict(list)
            for qc in range(3):
                for s_ in range(4):
                    for j_ in range(2):
                        g_ = 32 * qc + 32 + DELAY + 2 * s_ + j_
                        if g_ < NG:
                            sched[g_].append(prj(qc, s_, j_))
                            budget[g_] -= 0.53
                        else:
                            post_loop[g_ - NG].append(prj(qc, s_, j_))
            placed = []  # (group or -1 for prefix, order, chain, fn)
            for e, cost, chain, fn in sorted(units, key=lambda u: u[0]):
                g = min(e, NG - 1)
                while g >= 0 and budget[g] <= 1e-9:
                    g -= 1
                if g >= 0:
                    budget[g] -= cost
                placed.append([g, chain, fn])
            # re-bind chained units: sort each chain's slots, keep fn order
            by_chain = defaultdict(list)
            for i, (g, chain, fn) in enumerate(placed):
                if chain is not None:
                    by_chain[chain].append(i)
            for idxs in by_chain.values():
                slots = sorted(placed[i][0] for i in idxs)
                for i, s_ in zip(idxs, slots):
                    placed[i][0] = s_
            prefix_units = []
            for g, chain, fn in placed:
                if g < 0:
                    prefix_units.append(fn)
                else:
                    sched[g].append(fn)

            # ---- PE warm-up: junk matmuls on constants while the first x/w
            # DMAs are in flight, so the p-state ramp (0.65 -> 2.4 GHz over a
            # 3us busy streak) completes before the real prefix work starts
            junk = sp.tile([P, 2 * QW], F32, name="ps")
            for _ in range(26):
                nc.tensor.matmul(
                    junk[:, 0:P], ones128[:], ones128[:], start=True, stop=True
                )

            # ---- prefix: just enough to start the pipeline
            qk_half(0, 0, 0)
            qk_half(0, 0, 1)   # Q pair0 chunk0
            qk_half(2, 0, 0)
            qk_half(2, 0, 1)   # K pair0 chunk0
            pb0 = pp.tile([P, FQ], F32, name="pp")
            nc.tensor.matmul(pb0[:], ones_f[:], bv_s[:], start=True, stop=True)
            nc.vector.tensor_copy(bv_bcast[:], pb0[:])
            for fn in prefix_units:
                fn()

            for g, (qc, p, kt) in enumerate(seq):
                # av first (its exp finished long before this group's scores
                # dep) -- EXCEPT a chunk's first av, which waits on the
                # previous chunk's norm to free the po region and would clog
                # the PE wait-queue ahead of the scores
                av_first = g >= DELAY and seq[g - DELAY][2] != 0
                if av_first:
                    emit_av(g - DELAY, *seq[g - DELAY])
                emit_scores(g, qc, p, kt)
                for fn in sched.get(g, ()):
                    fn()
                if g >= DELAY and not av_first:
                    emit_av(g - DELAY, *seq[g - DELAY])
            for i, g in enumerate(range(NG - DELAY, NG)):
                emit_av(g, *seq[g])
                for fn in post_loop.get(i, ()):
                    fn()

    nc.compile()
    return nc


_CACHE = {}


def _get_nc():
    if "nc" not in _CACHE:
        _CACHE["nc"] = build()
    return _CACHE["nc"]


def make_in_maps(x, w_qkv, b_qkv, w_proj, b_proj):
    x = np.asarray(x, dtype=np.float32)
    w_qkv = np.asarray(w_qkv, dtype=np.float32)
    b_qkv = np.asarray(b_qkv, dtype=np.float32)
    w_proj = np.asarray(w_proj, dtype=np.float32)
    b_proj = np.asarray(b_proj, dtype=np.float32)
    in_maps = []
    for c in range(N_CORES):
        b, g = c // 4, c % 4
        f = slice(g * FQ, (g + 1) * FQ)
        fq = slice(g * FQ, (g + 1) * FQ)
        fk = slice(D + g * FQ, D + (g + 1) * FQ)
        fv = slice(2 * D + g * FQ, 2 * D + (g + 1) * FQ)
        in_maps.append(
            {
                "xt": np.ascontiguousarray(x[b].T),
                "wqk": np.ascontiguousarray(
                    np.concatenate([w_qkv[:, fq], w_qkv[:, fk]], axis=1)
                ),
                "wv": np.ascontiguousarray(w_qkv[:, fv]),
                "bqk": np.concatenate([b_qkv[fq], b_qkv[fk]]).reshape(2 * FQ, 1).copy(),
                "bv": b_qkv[fv].reshape(1, FQ).copy(),
                "wpr": np.ascontiguousarray(w_proj[f, :]),
                "bpr": (b_proj / 4.0).reshape(1, D).copy(),
            }
        )
    return in_maps


def assemble(results):
    out = np.empty((B, S, D), dtype=np.float32)
    for b in range(B):
        grp = [
            np.asarray(results[4 * b + i]["out"], dtype=np.float32)
            for i in range(4)
        ]
        out[b] = grp[0] + grp[1] + grp[2] + grp[3]
    return out


def kernel(x, w_qkv, b_qkv, w_proj, b_proj, num_heads=H, **_):
    in_maps = make_in_maps(x, w_qkv, b_qkv, w_proj, b_proj)
    res = run_bass_kernel_spmd(
        _get_nc(), in_maps, core_ids=list(range(N_CORES))
    )
    return assemble(res.results)
